# revision 6
# baseline (speedup 1.0000x reference)
"""GCN layer (segment-sum message passing) on 8 Trainium2 NeuronCores.

out = D_in^{-1/2} A D_out^{-1/2} X W + b, A given as an edge list.

Strategy (per the dst-sharding hint):
  - dst nodes sharded 12500/core across 8 cores; edges partitioned by dst core.
  - Inside a core, dst range is cut into 112 stripes of 112 nodes. Edges are
    bucketed by (stripe, src-quadrant); each bucket is padded to whole chunks
    of 128 edges. Chunk counts per bucket are the max over the 8 cores, so one
    SPMD program serves all cores (per-core variation lives in the data).
  - x is replicated per core as 4 quadrant tables of 25000 rows (dma_gather
    indices are int16). Per chunk, the 128 source rows are gathered from HBM
    (dma_gather, 4 SWDGE queues round-robin).
  - Aggregation per chunk is a PE matmul: psum[64f, 112d] += msgs[128e, 64f]^T
    @ P[128e, 112d], where P[e, j] = (iota[j] == dstoff[e]) * rsqrt(deg_out)
    is built in one fused DVE tensor_scalar op. Stripe psum is then added into
    a persistent SBUF accumulator at a static window offset.
  - Final phase per 128-dst block: psum2[128d, 64] = agg_blk^T @ W, then one
    fused DVE op applies rsqrt(deg_in) scaling and adds the bias.
All floating-point math runs on device; the host only does integer graph
restructuring (sharding/bucketing/padding) and array layout.
"""
import os
import sys

sys.path.insert(0, "/opt/trn_rl_repo")

import numpy as np

import concourse.bass as bass
import concourse.bacc as bacc
import concourse.mybir as mybir
from concourse.bass_utils import run_bass_kernel_spmd
from concourse.tile import TileContext

N_NODES = 100000
N_EDGES = 1200000
D = 64
NCORES = 8
NV = 16                          # virtual cores; 2 executions of 8 cores
                                 # (SWDGE sem budget caps one exec ~131K idxs)
PER = N_NODES // NV              # 6250 dst nodes per virtual core
STRIPE = 112                     # dst nodes per stripe (= onehot width)
NSTR = (PER + STRIPE - 1) // STRIPE   # 56 stripes (56*112 = 6272)
PERPAD = NSTR * STRIPE           # 6272
NBLK = PERPAD // 128             # 49 output blocks of 128 dsts
NQ = 4                           # src quadrant tables
QSIZE = N_NODES // NQ            # 25000 rows per table (int16-indexable)
CHK = 128                        # edges per chunk
CALL_CHUNKS = 8                  # chunks per dma_gather call (1024 idx, HW-validated)
AGGW = PERPAD + 128              # agg free width incl. window spill margin

F32 = mybir.dt.float32
I16 = mybir.dt.int16

LAST_EXEC_NS = None


def _prep(edge_index):
    """Integer-only host prep: shard, bucket, pad, lay out streams."""
    src = edge_index[0].astype(np.int64)
    dst = edge_index[1].astype(np.int64)
    deg_out = np.bincount(src, minlength=N_NODES)
    deg_in = np.bincount(dst, minlength=N_NODES)

    core = dst // PER
    dstl = dst - core * PER
    g = dstl // STRIPE
    q = src // QSIZE
    srcl = (src - q * QSIZE).astype(np.int64)

    # per-(vcore, stripe, quadrant) bucket sizes
    key = (core * NSTR + g) * NQ + q
    cnt = np.bincount(key, minlength=NV * NSTR * NQ).reshape(NV, NSTR, NQ)
    K = -(-cnt.max(axis=0) // CHK)          # chunks per (stripe, quadrant) cell
    K = K.astype(np.int64)

    # global chunk ids in (g, q, k) order; per-quadrant stream positions
    cell_chunk_base = np.zeros((NSTR, NQ), np.int64)
    qpos = np.zeros((NSTR, NQ), np.int64)
    nchunks = 0
    qlen = np.zeros(NQ, np.int64)
    for gi in range(NSTR):
        for qi in range(NQ):
            cell_chunk_base[gi, qi] = nchunks
            nchunks += K[gi, qi]
            qpos[gi, qi] = qlen[qi]
            qlen[qi] += K[gi, qi]

    # gather calls: per quadrant, slices of CALL_CHUNKS chunks
    calls = []            # (q, stream_chunk_start, nchunks, col_off)
    col_off = 0
    stream_call_base = []  # per q: array mapping stream chunk -> (call idx, slot)
    for qi in range(NQ):
        s = 0
        while s < qlen[qi]:
            c = min(CALL_CHUNKS, qlen[qi] - s)
            calls.append((qi, s, int(c), col_off))
            col_off += int(c) * CHK // 16
            s += c
    totcols = col_off

    # map global chunk -> (call index, slot in call's msgs tile)
    call_of_stream = {}
    for ci, (qi, s, c, _) in enumerate(calls):
        for j in range(c):
            call_of_stream[(qi, s + j)] = (ci, j)
    chunk_call = np.zeros((nchunks, 2), np.int64)
    for gi in range(NSTR):
        for qi in range(NQ):
            for k in range(int(K[gi, qi])):
                gc = cell_chunk_base[gi, qi] + k
                chunk_call[gc] = call_of_stream[(qi, qpos[gi, qi] + k)]

    # per-edge slot assignment (vectorized): order edges by (core, g, q),
    # then within-bucket rank gives (k, p).
    order = np.lexsort((q, g, core))
    so_core, so_g, so_q = core[order], g[order], q[order]
    so_srcl, so_dstl = srcl[order], dstl[order]
    so_src = src[order]
    okey = (so_core * NSTR + so_g) * NQ + so_q
    # rank within bucket
    bucket_start = np.searchsorted(okey, np.arange(NV * NSTR * NQ), side="left")
    rank = np.arange(len(order)) - bucket_start[okey]
    k_of = rank // CHK
    p_of = rank % CHK
    gchunk = cell_chunk_base[so_g, so_q] + k_of

    # stream slot position for gather index layout
    spos = (qpos[so_g, so_q] + k_of)  # stream chunk within quadrant

    # per-vcore output arrays
    cores_data = []
    for c in range(NV):
        m = so_core == c
        gidx_streams = [np.zeros(int(qlen[qi]) * CHK, np.int16) for qi in range(NQ)]
        dstoff = np.full((CHK, nchunks), -1, np.int16)
        dgo = np.ones((CHK, nchunks), np.int16)
        sq, ssl, sdl = so_q[m], so_srcl[m], so_dstl[m]
        sg, sp, sgc, ssp = so_g[m], p_of[m], gchunk[m], spos[m]
        sdeg = deg_out[so_src[m]]
        for qi in range(NQ):
            mq = sq == qi
            gidx_streams[qi][ssp[mq] * CHK + sp[mq]] = ssl[mq].astype(np.int16)
        dstoff[sp, sgc] = (sdl - sg * STRIPE).astype(np.int16)
        dgo[sp, sgc] = np.minimum(sdeg, 32000).astype(np.int16)

        # wrap gather indices into the [128, totcols] int16 layout, per call
        gidx = np.zeros((128, totcols), np.int16)
        for (qi, s, cc, coff) in calls:
            seq = gidx_streams[qi][s * CHK:(s + cc) * CHK]
            wr = seq.reshape(-1, 16).T  # [16, cc*8]
            gidx[:, coff:coff + cc * CHK // 16] = np.tile(wr, (8, 1))

        # dgi layout: [p, k] with d = 128k + p
        base = c * PER
        dgi2 = np.ones((128, NBLK), np.int16)
        d_arr = np.arange(PERPAD)
        p_arr = d_arr % 128
        k_arr = d_arr // 128
        dv = np.ones(PERPAD, np.int64)
        dv[d_arr < PER] = deg_in[base:base + PER]
        dgi2[p_arr, k_arr] = np.minimum(np.maximum(dv, 0), 32000).astype(np.int16)

        cores_data.append({
            "gidx": gidx,
            "dstoff": dstoff.astype(np.int16),
            "dgo": dgo,
            "dgi": dgi2,
        })

    struct = {
        "K": K, "nchunks": int(nchunks), "calls": calls, "totcols": int(totcols),
        "cell_chunk_base": cell_chunk_base, "chunk_call": chunk_call,
    }
    return struct, cores_data


def _build(struct):
    K = struct["K"]
    nchunks = struct["nchunks"]
    calls = struct["calls"]
    totcols = struct["totcols"]
    cell_chunk_base = struct["cell_chunk_base"]
    chunk_call = struct["chunk_call"]

    nc = bacc.Bacc("TRN2", target_bir_lowering=False, num_swdge_queues=4)
    t_xq = [nc.declare_dram_parameter(f"xq{i}", [QSIZE, D], F32, isOutput=False)
            for i in range(NQ)]
    t_gidx = nc.declare_dram_parameter("gidx", [128, totcols], I16, isOutput=False)
    t_dstoff = nc.declare_dram_parameter("dstoff", [128, nchunks], I16, isOutput=False)
    t_dgo = nc.declare_dram_parameter("dgo", [128, nchunks], I16, isOutput=False)
    t_dgi = nc.declare_dram_parameter("dgi", [128, NBLK], I16, isOutput=False)
    t_w = nc.declare_dram_parameter("w", [D, D], F32, isOutput=False)
    t_bb = nc.declare_dram_parameter("bb", [128, D], F32, isOutput=False)
    t_out = nc.declare_dram_parameter("out", [PERPAD, D], F32, isOutput=True)

    with TileContext(nc) as tc:
        with (
            tc.tile_pool(name="const", bufs=1) as cp,
            tc.tile_pool(name="msgs", bufs=6) as mp,
            tc.tile_pool(name="oh", bufs=8) as ohp,
            tc.tile_pool(name="psg", bufs=5, space="PSUM") as psg,
            tc.tile_pool(name="psf", bufs=2, space="PSUM") as psf,
        ):
            gidx_sb = cp.tile([128, totcols], I16)
            nc.sync.dma_start(out=gidx_sb[:], in_=t_gidx[:])
            dstoff_i = cp.tile([128, nchunks], I16)
            nc.sync.dma_start(out=dstoff_i[:], in_=t_dstoff[:])
            dgo_i = cp.tile([128, nchunks], I16)
            nc.sync.dma_start(out=dgo_i[:], in_=t_dgo[:])
            dgi_i = cp.tile([128, NBLK], I16)
            nc.sync.dma_start(out=dgi_i[:], in_=t_dgi[:])
            w_sb = cp.tile([D, D], F32)
            nc.sync.dma_start(out=w_sb[:], in_=t_w[:])
            bb_sb = cp.tile([128, D], F32)
            nc.sync.dma_start(out=bb_sb[:], in_=t_bb[:])

            # s_out per slot, s_in per (p, blk): rsqrt(max(deg, 1))
            dstoff_f = cp.tile([128, nchunks], F32)
            nc.vector.tensor_copy(dstoff_f[:], dstoff_i[:])
            sout = cp.tile([128, nchunks], F32)
            nc.vector.tensor_copy(sout[:], dgo_i[:])
            nc.vector.tensor_scalar(sout[:], sout[:], 1.0, None,
                                    mybir.AluOpType.max)
            nc.scalar.activation(sout[:], sout[:],
                                 mybir.ActivationFunctionType.Sqrt)
            nc.vector.reciprocal(sout[:], sout[:])
            sgi = cp.tile([128, NBLK], F32)
            nc.vector.tensor_copy(sgi[:], dgi_i[:])
            nc.vector.tensor_scalar(sgi[:], sgi[:], 1.0, None,
                                    mybir.AluOpType.max)
            nc.scalar.activation(sgi[:], sgi[:],
                                 mybir.ActivationFunctionType.Sqrt)
            nc.vector.reciprocal(sgi[:], sgi[:])

            iota_i = cp.tile([128, STRIPE], mybir.dt.int32)
            nc.gpsimd.iota(iota_i[:], pattern=[[1, STRIPE]], base=0,
                           channel_multiplier=0)
            iota_f = cp.tile([128, STRIPE], F32)
            nc.vector.tensor_copy(iota_f[:], iota_i[:])

            agg = cp.tile([D, AGGW], F32)
            nc.vector.memset(agg[:], 0.0)

            msgs_tiles = {}
            emit_count = [0]

            def get_call_tile(ci):
                if ci not in msgs_tiles:
                    qi, s, cc, coff = calls[ci]
                    t = mp.tile([128, cc, D], F32, tag="msgs")
                    # queue follows Pool-DMA emission order so Tile's DMASW
                    # lane round-robin (8 lanes) stays queue-consistent
                    nc.gpsimd.dma_gather(
                        t[:], t_xq[qi][:],
                        gidx_sb[:, coff:coff + cc * CHK // 16],
                        cc * CHK, cc * CHK, D,
                        single_packet=True, queue_num=emit_count[0] % 4,
                    )
                    emit_count[0] += 1
                    msgs_tiles[ci] = t
                return msgs_tiles[ci]

            for gi in range(NSTR):
                stripe_chunks = []
                for qi in range(NQ):
                    for k in range(int(K[gi, qi])):
                        stripe_chunks.append(int(cell_chunk_base[gi, qi] + k))
                if not stripe_chunks:
                    continue
                ps = psg.tile([D, STRIPE], F32)
                for i, gc in enumerate(stripe_chunks):
                    ci, slot = int(chunk_call[gc, 0]), int(chunk_call[gc, 1])
                    mt = get_call_tile(ci)
                    P = ohp.tile([128, STRIPE], F32, tag="oh")
                    nc.vector.tensor_scalar(
                        P[:], iota_f[:], dstoff_f[:, gc:gc + 1],
                        sout[:, gc:gc + 1],
                        mybir.AluOpType.is_equal, mybir.AluOpType.mult,
                    )
                    nc.tensor.matmul(ps[:], mt[:, slot, :], P[:],
                                     start=(i == 0),
                                     stop=(i == len(stripe_chunks) - 1))
                w0 = gi * STRIPE
                nc.vector.tensor_tensor(
                    out=agg[:, w0:w0 + STRIPE], in0=agg[:, w0:w0 + STRIPE],
                    in1=ps[:], op=mybir.AluOpType.add,
                )

            out_sb = cp.tile([128, NBLK * D], F32)
            for k in range(NBLK):
                ps2 = psf.tile([128, D], F32)
                nc.tensor.matmul(ps2[:], agg[:, k * 128:(k + 1) * 128],
                                 w_sb[:], start=True, stop=True)
                nc.vector.scalar_tensor_tensor(
                    out=out_sb[:, k * D:(k + 1) * D], in0=ps2[:],
                    scalar=sgi[:, k:k + 1], in1=bb_sb[:],
                    op0=mybir.AluOpType.mult, op1=mybir.AluOpType.add,
                )
            nc.sync.dma_start(
                out=t_out[:].rearrange("(p k) f -> p (k f)", p=128),
                in_=out_sb[:],
            )

    nc.finalize()
    return nc


def kernel(**inputs):
    global LAST_EXEC_NS
    x = np.ascontiguousarray(np.asarray(inputs["x"], dtype=np.float32))
    edge_index = np.asarray(inputs["edge_index"]).astype(np.int64)
    W = np.ascontiguousarray(np.asarray(inputs["W"], dtype=np.float32))
    b = np.asarray(inputs["b"], dtype=np.float32).reshape(-1)

    struct, cores_data = _prep(edge_index)
    nc = _build(struct)

    bb = np.tile(b[None, :], (128, 1)).astype(np.float32)
    xqs = {f"xq{i}": np.ascontiguousarray(x[i * QSIZE:(i + 1) * QSIZE])
           for i in range(NQ)}
    in_maps = []
    for c in range(NV):
        m = dict(xqs)
        m["gidx"] = cores_data[c]["gidx"]
        m["dstoff"] = cores_data[c]["dstoff"]
        m["dgo"] = cores_data[c]["dgo"]
        m["dgi"] = cores_data[c]["dgi"]
        m["w"] = W
        m["bb"] = bb
        in_maps.append(m)

    results = [None] * NV
    LAST_EXEC_NS = 0
    for half in range(NV // NCORES):
        batch = in_maps[half * NCORES:(half + 1) * NCORES]
        if os.environ.get("GCN_SIM"):
            import concourse.bass_interp as bass_interp
            sim = bass_interp.MultiCoreSim(nc, NCORES)
            for c in range(NCORES):
                for k, v in batch[c].items():
                    sim.cores[c].tensor(k)[:] = v
            sim.simulate()
            for c in range(NCORES):
                results[half * NCORES + c] = {
                    "out": np.array(sim.cores[c].mem_tensor("out"))}
            LAST_EXEC_NS = None
        else:
            trace = bool(os.environ.get("GCN_TRACE"))
            res = run_bass_kernel_spmd(nc, batch, list(range(NCORES)),
                                       trace=trace)
            if LAST_EXEC_NS is not None and res.exec_time_ns is not None:
                LAST_EXEC_NS += res.exec_time_ns
            else:
                LAST_EXEC_NS = None
            for c in range(NCORES):
                results[half * NCORES + c] = res.results[c]

    outs = []
    for v in range(NV):
        o = results[v]["out"]  # [6272, 64], row r = p*49 + k, d = 128k+p
        o = o.reshape(128, NBLK, D).transpose(1, 0, 2).reshape(PERPAD, D)
        outs.append(o[:PER])
    return np.concatenate(outs, axis=0).astype(np.float32)
